# revision 8
# baseline (speedup 1.0000x reference)
"""Chebyshev graph-conv kernel for Trainium2 (8 NeuronCores, SPMD).

Math: out[b,o,m,t] = sum_{k,c,n} T[k,n,m] * x[b,c,n,t] * Theta[k,c,o]
with T the Chebyshev polynomials of the normalized adjacency (n=24, K=3).

The whole operator collapses into a single 768x768 matrix
    W[(c,n),(o,m)] = sum_k Theta[k,c,o] * T[k,n,m]
applied per batch element to x[b] viewed as (c*n, t) = (768, 512):
    out[b](o*24+m, t) = W.T-contract over rows -> exactly one matmul chain.

W is tiny (2.25 MB) and computed on host from adj/Theta; x is read once and
out written once (memory-roofline). Data-parallel over batch: 64 -> 8 per core.

Per core: 8 batch elements; for each, 6x6 [128,128]x[128,512] float32r
matmuls accumulated in PSUM (float32r = full-rate FP22 multiply, fp32 accum).
"""

import numpy as np

import concourse.bass as bass
import concourse.mybir as mybir
from concourse import bacc, tile
from concourse.bass_utils import run_bass_kernel_spmd

N_CORES = 8
B, C, NV, T = 64, 32, 24, 512
K = 3
O = 32
CN = C * NV   # 768 contraction rows
OM = O * NV   # 768 output rows
BP = B // N_CORES  # 8 batch elements per core
P = 128
NBLK = CN // P  # 6

_compiled_nc = None
last_result = None  # BassKernelResults from the most recent run (for test.py)


def _build_nc():
    f32 = mybir.dt.float32
    f32r = mybir.dt.float32r
    nc = bacc.Bacc("TRN2", target_bir_lowering=False, debug=False,
                   num_devices=N_CORES)
    xs = nc.dram_tensor("xs", [BP, CN, T], f32, kind="ExternalInput")
    w = nc.dram_tensor("w", [CN, OM], f32, kind="ExternalInput")
    out = nc.dram_tensor("out", [BP, OM, T], f32, kind="ExternalOutput")

    wr = w[:].rearrange("(i p) m -> p i m", p=P).bitcast(f32r)

    with tile.TileContext(nc) as tc:
        with (
            tc.tile_pool(name="wpool", bufs=1) as wpool,
            tc.tile_pool(name="xpool", bufs=5) as xpool,
            tc.tile_pool(name="opool", bufs=4) as opool,
            tc.tile_pool(name="psum", bufs=8, space="PSUM") as psum_pool,
        ):
            # W as 6 chunks of [128 (cn), 768 (om)], all in one SBUF tile.
            # Tiles are float32r (bit-identical to f32; matmul runs at full
            # rate with FP22 multiply) — BIR verifier requires the producer
            # of an fp32r matmul operand to be typed fp32r.
            # Loads go on the Sync HWDGE ring; stores on the Scalar HWDGE
            # ring so stores never head-of-line-block loads. W and the first
            # batch's x are loaded chunk-wise so the first matmul only waits
            # for chunk 0 of each (~0.6 MB) instead of the full 3.75 MB.
            wt = wpool.tile([P, NBLK, OM], f32r)
            xt0 = xpool.tile([P, NBLK, T], f32r)
            xr0 = xs[0].rearrange("(i p) t -> p i t", p=P).bitcast(f32r)
            for i in range(NBLK):
                nc.sync.dma_start(wt[:, i, :], wr[:, i, :])
                nc.sync.dma_start(xt0[:, i, :], xr0[:, i, :])

            for b in range(BP):
                if b == 0:
                    xt = xt0
                else:
                    # Alternate x loads between the Sync HWDGE queue and the
                    # GpSimd SWDGE queue: loads then own 2 of the 3 active DMA
                    # queues, so the packet-round-robin gives them ~2/3 of HBM
                    # bandwidth and they run ahead of the store stream.
                    xt = xpool.tile([P, NBLK, T], f32r, tag="xt0")
                    xr = xs[b].rearrange("(i p) t -> p i t", p=P).bitcast(f32r)
                    eng = nc.sync if b % 2 == 0 else nc.gpsimd
                    eng.dma_start(xt[:, 0:3, :], xr[:, 0:3, :])
                    eng.dma_start(xt[:, 3:6, :], xr[:, 3:6, :])
                ot = opool.tile([P, NBLK, T], f32)
                orr = out[b].rearrange("(j p) t -> p j t", p=P)
                for j in range(NBLK):
                    ps = psum_pool.tile([P, T], f32)
                    for i in range(NBLK):
                        nc.tensor.matmul(
                            ps[:],
                            wt[:, i, j * P:(j + 1) * P],
                            xt[:, i, :],
                            start=(i == 0),
                            stop=(i == NBLK - 1),
                        )
                    nc.vector.tensor_copy(ot[:, j, :], ps[:])
                    nc.scalar.dma_start(orr[:, j, :], ot[:, j, :])

    nc.compile()
    return nc


def _combined_operator(adj: np.ndarray, Theta: np.ndarray) -> np.ndarray:
    """W[(c,n),(o,m)] = sum_k Theta[k,c,o] * T[k,n,m], fp32, shape (768,768)."""
    adj = adj.astype(np.float32)
    d = adj.sum(axis=1)
    d_inv_sqrt = np.where(d > 0, 1.0 / np.sqrt(d), 0.0).astype(np.float32)
    L = (adj * d_inv_sqrt[None, :]).T * d_inv_sqrt[None, :]
    Ts = [np.eye(NV, dtype=np.float32), L.astype(np.float32)]
    for _ in range(2, K):
        Ts.append((2.0 * L @ Ts[-1] - Ts[-2]).astype(np.float32))
    Tcheb = np.stack(Ts[:K])  # (K, n, m)
    W = np.einsum("kco,knm->cnom", Theta.astype(np.float32), Tcheb)
    return np.ascontiguousarray(W.reshape(CN, OM), dtype=np.float32)


def kernel(x: np.ndarray, adj: np.ndarray, Theta: np.ndarray) -> np.ndarray:
    global _compiled_nc, last_result
    if _compiled_nc is None:
        _compiled_nc = _build_nc()
    nc = _compiled_nc

    W = _combined_operator(adj, Theta)
    # x: (64, 32, 24, 512) -> per-core shard [8, 768, 512]
    xf = np.ascontiguousarray(x, dtype=np.float32).reshape(B, CN, T)
    in_maps = [
        {"xs": np.ascontiguousarray(xf[c * BP:(c + 1) * BP]), "w": W}
        for c in range(N_CORES)
    ]
    res = run_bass_kernel_spmd(nc, in_maps, core_ids=list(range(N_CORES)))
    last_result = res
    out = np.concatenate([r["out"] for r in res.results], axis=0)
    return np.ascontiguousarray(out.reshape(B, O, NV, T))
